# revision 19
# baseline (speedup 1.0000x reference)
"""Trainium2 Bass kernel for nn_CompletePipeline_5772436046193.

Pipeline per image: bicubic /2 downsample -> per-channel gains -> 5x5 bilateral
-> gamma -> conv5x5(3->64)+relu -> conv3x3(64->64)+relu -> bilinear x2 upsample
-> conv3x3(64->3).  Outputs (x_sr[4,3,512,512], x_lr[4,3,256,256]).

Sharding: 8 cores = 4 images x 2 vertical halves.  Bottom halves run the same
program on vertically flipped inputs with dy-flipped conv kernels.

Per-core dataflow (all "top half" orientation):
  - down2-W on DVE, down2-H as PE band-matrix matmul (gains folded in)
  - bilateral input staged to DRAM with reflect borders, re-gathered as a
    [128 rows, 5dy, 3c, 260] replica tile; 24 taps on DVE+ACT
  - gamma on ACT (Ln, Exp); x_lr written; bf16 copy staged for conv1
  - conv1: K=15 (ci x dy) matmuls, 5 dx taps, M=128 duplicated output
  - conv2: 3x3 via K=128 two-tap packing (shifted duplicate in partitions
    64..127), 3 pair + 3 single matmul streams
  - upsample+conv3 fused: 4 parity 3x3 convs on the LR grid, M=12, same
    two-tap packing; parity-planar DRAM layout, interleaved on host
  - exact border handling: row-0 / col-0 / col-511 fix matmuls
"""
import sys
import numpy as np

sys.path.insert(0, "/opt/trn_rl_repo")

import concourse.bass as bass  # noqa: E402
import concourse.mybir as mybir  # noqa: E402
import concourse.tile as tile  # noqa: E402
from concourse import bacc  # noqa: E402
from concourse import bass_utils  # noqa: E402
import bass_rust  # noqa: E402

F32 = mybir.dt.float32
BF16 = mybir.dt.bfloat16
AF = mybir.ActivationFunctionType
ALU = mybir.AluOpType

CUBIC = np.array([-0.09375, 0.59375, 0.59375, -0.09375], np.float32)
WL = 256          # LR width
SLAB_H = 270      # clamp row + image rows 0..268
N_LR = 134        # LR rows produced by down2
N_BL = 132        # bilateral/gamma rows (x_lr rows 0..131)
HCHUNKS = [(0, 62, 0, 127), (62, 62, 124, 127), (124, 10, 248, 22)]  # (j0, nj, s0, ns)
BCHUNKS = [(0, 128), (128, 4)]
BANDS = [0, 32, 64, 96]
BW = 32


def _ap(base, offset_el, dims):
    """Build a custom access pattern on base tensor's AP."""
    c = base.copy()
    c.offset = int(base.offset) + int(offset_el)
    c.ap = bass_rust.VecI64Pair([[int(s), int(n)] for s, n in dims])
    return c


# --------------------------------------------------------------------------
# host weight packing
# --------------------------------------------------------------------------

def _upsample_mat():
    """U[p, t(-1..1), d(-1..1)]: coeff of y[j+d] in up[2j+p+t]."""
    U = np.zeros((2, 3, 3), np.float32)
    for p in range(2):
        for ti, t in enumerate((-1, 0, 1)):
            h = p + t
            if h % 2 == 0:
                jj = h // 2
                U[p, ti, 1 + jj] += 0.75
                U[p, ti, jj] += 0.25
            else:
                jj = (h - 1) // 2
                U[p, ti, 1 + jj] += 0.75
                U[p, ti, 2 + jj] += 0.25
    return U


def host_weights(gains, w1, b1, w2, b2, w3, b3, flip):
    """Pack all weight tensors for one core orientation."""
    w1 = np.asarray(w1, np.float32)
    w2 = np.asarray(w2, np.float32)
    w3 = np.asarray(w3, np.float32)
    if flip:
        w1 = w1[:, :, ::-1, :]
        w2 = w2[:, :, ::-1, :]
        w3 = w3[:, :, ::-1, :]
    gains = np.asarray(gains, np.float32)
    b1 = np.asarray(b1, np.float32)
    b2 = np.asarray(b2, np.float32)
    b3 = np.asarray(b3, np.float32)
    out = {}

    # down2-H band matrices, gains folded: dh[chunk, c, K=127, M=62]
    dh = np.zeros((3, 3, 127, 62), np.float32)
    for ci, (j0, nj, s0, ns) in enumerate(HCHUNKS):
        for j in range(nj):
            for k in range(4):
                r = 2 * j + k
                if r < ns:
                    dh[ci, :, r, j] += CUBIC[k]
    dh *= gains.reshape(1, 3, 1, 1)
    out["dh"] = dh

    # conv1: w1p[dx, (c,dy)=15, 128] duplicated co
    w1p = np.zeros((5, 15, 128), np.float32)
    for dx in range(5):
        for c in range(3):
            for dy in range(5):
                w1p[dx, c * 5 + dy, 0:64] = w1[:, c, dy, dx]
                w1p[dx, c * 5 + dy, 64:128] = w1[:, c, dy, dx]
    out["w1p"] = w1p.astype(np.float32)
    out["b1d"] = np.concatenate([b1, b1]).reshape(128, 1)

    # conv2 pairs: taps (dy,0)+(dy,1) stacked K=128; singles (dy,2) K=64
    w2p = np.zeros((3, 128, 128), np.float32)
    w2s = np.zeros((3, 64, 128), np.float32)
    for dy in range(3):
        for ci in range(64):
            w2p[dy, ci, 0:64] = w2[:, ci, dy, 0]
            w2p[dy, ci, 64:128] = w2[:, ci, dy, 0]
            w2p[dy, 64 + ci, 0:64] = w2[:, ci, dy, 1]
            w2p[dy, 64 + ci, 64:128] = w2[:, ci, dy, 1]
            w2s[dy, ci, 0:64] = w2[:, ci, dy, 2]
            w2s[dy, ci, 64:128] = w2[:, ci, dy, 2]
    out["w2p"] = w2p
    out["w2s"] = w2s
    out["b2d"] = np.concatenate([b2, b2]).reshape(128, 1)

    # fused upsample+conv3 weights
    U = _upsample_mat()
    Wf = np.einsum("oitj,pty,qjx->pqoiyx", w3, U, U).astype(np.float32)
    # main: M=12 ordered (co,py,px) -> m = co*4+py*2+px
    w3p = np.zeros((3, 128, 12), np.float32)
    w3s = np.zeros((3, 64, 12), np.float32)
    for dy in range(3):
        for co in range(3):
            for py in range(2):
                for px in range(2):
                    m = co * 4 + py * 2 + px
                    w3p[dy, 0:64, m] = Wf[py, px, co, :, dy, 0]
                    w3p[dy, 64:128, m] = Wf[py, px, co, :, dy, 1]
                    w3s[dy, :, m] = Wf[py, px, co, :, dy, 2]
    out["w3p"] = w3p
    out["w3s"] = w3s
    b3m = np.zeros((12, 1), np.float32)
    for co in range(3):
        b3m[co * 4:co * 4 + 4] = b3[co]
    out["b3m"] = b3m

    # row-0 fix: 12 taps, M=6 ordered (px,co): m = px*3+co
    # psum = generic(9 taps over y2r) - sum_dx M_px[dx] @ y2row0
    w3r0 = np.zeros((12, 64, 6), np.float32)
    i = 0
    for dy in range(3):
        for dx in range(3):
            for px in range(2):
                for co in range(3):
                    w3r0[i, :, px * 3 + co] = Wf[0, px, co, :, dy, dx]
            i += 1
    for dx in range(3):
        for px in range(2):
            Mpx = np.einsum("oit,t->oi", w3[:, :, 0, :], U[px, :, dx])
            for co in range(3):
                w3r0[i, :, px * 3 + co] = -Mpx[co]
        i += 1
    out["w3r0"] = w3r0
    b3r = np.zeros((6, 1), np.float32)
    for px in range(2):
        for co in range(3):
            b3r[px * 3 + co] = b3[co]
    out["b3r"] = b3r

    # col fixes: 16 taps each, M=6 ordered (py,co): m = py*3+co
    def col_fix(side):
        Wt = np.zeros((16, 64, 6), np.float32)
        i = 0
        px = 0 if side == 0 else 1
        for dy in range(3):
            for dx in range(3):
                for py in range(2):
                    for co in range(3):
                        Wt[i, :, py * 3 + co] = Wf[py, px, co, :, dy, dx]
                i += 1
        wcol = w3[:, :, :, 0] if side == 0 else w3[:, :, :, 2]
        for dy in range(3):
            for py in range(2):
                Mh = np.einsum("oit,t->oi", wcol, U[py, :, dy])
                for co in range(3):
                    Wt[i, :, py * 3 + co] = -Mh[co]
            i += 1
        for dx in range(3):
            Mpx = np.einsum("oit,t->oi", w3[:, :, 0, :], U[px, :, dx])
            for co in range(3):
                Wt[i, :, co] = -Mpx[co]
            i += 1
        wc = w3[:, :, 0, 0] if side == 0 else w3[:, :, 0, 2]
        for co in range(3):
            Wt[i, :, co] = wc[co]
        return Wt

    out["w3c0"] = col_fix(0)
    out["w3c1"] = col_fix(1)
    b3c = np.zeros((6, 1), np.float32)
    for py in range(2):
        for co in range(3):
            b3c[py * 3 + co] = b3[co]
    out["b3c"] = b3c
    return out


def make_slab(x_img, half):
    if half == 1:
        x_img = x_img[:, ::-1, :]
    return np.ascontiguousarray(
        np.concatenate([x_img[:, :1], x_img[:, :269]], axis=1), dtype=np.float32)


# --------------------------------------------------------------------------
# program build
# --------------------------------------------------------------------------

SPATIAL_LNW = None


def _spatial_lnw():
    o = np.arange(-2.0, 3.0, dtype=np.float32)
    xx, yy = np.meshgrid(o, o, indexing="ij")
    return (-(xx ** 2 + yy ** 2) / 2.0).astype(np.float32)


def build_program(sigma_color, gamma):
    nc = bacc.Bacc("TRN2", target_bir_lowering=False, debug=False,
                   enable_asserts=False, num_devices=8)
    inv2sig = 1.0 / (2.0 * float(sigma_color) ** 2)
    invgam = 1.0 / float(gamma)
    lnw = _spatial_lnw()

    def din(name, shape, dt=F32):
        return nc.dram_tensor(name, list(shape), dt, kind="ExternalInput").ap()

    slab = din("slab", [3, SLAB_H, 512])
    dh = din("dh", [3, 3, 127, 62])
    w1p = din("w1p", [5, 15, 128])
    b1d = din("b1d", [128, 1])
    w2p = din("w2p", [3, 128, 128])
    w2s = din("w2s", [3, 64, 128])
    b2d = din("b2d", [128, 1])
    w3p = din("w3p", [3, 128, 12])
    w3s = din("w3s", [3, 64, 12])
    b3m = din("b3m", [12, 1])
    w3r0 = din("w3r0", [12, 64, 6])
    b3r = din("b3r", [6, 1])
    w3c0 = din("w3c0", [16, 64, 6])
    w3c1 = din("w3c1", [16, 64, 6])
    b3c = din("b3c", [6, 1])

    xlr_o = nc.dram_tensor("xlr", [3, 128, 256], F32, kind="ExternalOutput").ap()
    xsr_o = nc.dram_tensor("xsr", [3, 2, 2, 128, 256], F32, kind="ExternalOutput").ap()

    with tile.TileContext(nc) as tc:
        _body(nc, tc, locals(), inv2sig, invgam, lnw)
    nc.compile()
    return nc


def _body(nc, tc, T, inv2sig, invgam, lnw):
    slab, dh = T["slab"], T["dh"]
    xlr_o, xsr_o = T["xlr_o"], T["xsr_o"]

    with (
        tc.tile_pool(name="dram", bufs=1, space="DRAM") as pd,
        tc.tile_pool(name="const", bufs=1) as pk,
    ):
        stg1 = pd.tile([3, 136, 260], F32)      # bilateral input staging
        stg2 = pd.tile([3, 134, 260], F32)     # conv1 input staging (padded)

        # ---------------- stage A: downsample ----------------
        with (
            tc.tile_pool(name="pA", bufs=2) as pA,
            tc.tile_pool(name="psA", bufs=2, space="PSUM") as psA,
        ):
            dht = pk.tile([127, 3 * 3 * 62], F32)
            nc.sync.dma_start(out=dht, in_=_ap(dh, 0, [[62, 127], [23622, 3], [7874, 3], [1, 62]]))
            for (j0, nj, s0, ns) in HCHUNKS:
                sl = pA.tile([128, 3, 514], F32, tag="slab")
                nc.sync.dma_start(
                    out=sl[:ns, :, 1:513],
                    in_=_ap(slab, s0 * 512, [[512, ns], [SLAB_H * 512, 3], [1, 512]]))
                nc.vector.tensor_copy(out=sl[:ns, :, 0:1], in_=sl[:ns, :, 1:2])
                nc.vector.tensor_copy(out=sl[:ns, :, 513:514], in_=sl[:ns, :, 512:513])
                xw = pA.tile([128, 3, 256], F32, tag="xw")
                for k in range(4):
                    src = _ap(sl, k, [[sl.ap[0][0], ns], [514, 3], [2, 256]])
                    if k == 0:
                        nc.vector.tensor_scalar(
                            out=xw[:ns], in0=src, scalar1=float(CUBIC[0]),
                            scalar2=None, op0=ALU.mult)
                    else:
                        nc.vector.scalar_tensor_tensor(
                            out=xw[:ns], in0=src, scalar=float(CUBIC[k]),
                            in1=xw[:ns], op0=ALU.mult, op1=ALU.add)
                pdm = psA.tile([62, 3, 512], F32, tag="pd")
                for c in range(3):
                    lhsT = _ap(dht, (HCHUNKS.index((j0, nj, s0, ns)) * 3 + c) * 62,
                               [[dht.ap[0][0], ns], [1, nj]])
                    nc.tensor.matmul(pdm[:nj, c, 0:256], lhsT, xw[:ns, c, :],
                                     start=True, stop=True)
                xwb = pA.tile([62, 3, 260], F32, tag="xwb")
                nc.scalar.copy(out=xwb[:nj, :, 2:258], in_=pdm[:nj, :, 0:256])
                # reflect cols: c-2<-c2(t4), c-1<-c1(t3), c256<-c254(t256), c257<-c253(t255)
                nc.vector.tensor_copy(out=xwb[:nj, :, 0:1], in_=xwb[:nj, :, 4:5])
                nc.vector.tensor_copy(out=xwb[:nj, :, 1:2], in_=xwb[:nj, :, 3:4])
                nc.vector.tensor_copy(out=xwb[:nj, :, 258:259], in_=xwb[:nj, :, 256:257])
                nc.vector.tensor_copy(out=xwb[:nj, :, 259:260], in_=xwb[:nj, :, 255:256])
                nc.sync.dma_start(
                    out=_ap(stg1, (2 + j0) * 260, [[260, nj], [136 * 260, 3], [1, 260]]),
                    in_=xwb[:nj])
                if j0 == 0:
                    # reflect rows: stg1 row0 = LR row2, row1 = LR row1
                    nc.sync.dma_start(
                        out=_ap(stg1, 0, [[260, 1], [136 * 260, 3], [1, 260]]),
                        in_=xwb[2:3])
                    nc.sync.dma_start(
                        out=_ap(stg1, 260, [[260, 1], [136 * 260, 3], [1, 260]]),
                        in_=xwb[1:2])

        # ---------------- stage B: bilateral + gamma ----------------
        with tc.tile_pool(name="pB", bufs=2) as pB:
            zt = pk.tile([2, 3 * 260], F32)
            nc.vector.memset(zt, 0.0)
            nc.sync.dma_start(
                out=_ap(stg2, 0, [[260, 2], [134 * 260, 3], [1, 260]]), in_=zt)
            lnvals = sorted({float(lnw[a, b]) for a in range(5) for b in range(5)})
            lncol = {v: i for i, v in enumerate(lnvals)}
            lnwt = pk.tile([128, len(lnvals)], F32)
            for v, i in lncol.items():
                nc.vector.memset(lnwt[:, i:i + 1], v)
            for (j0, nj) in BCHUNKS:
                g = pB.tile([128, 5, 3, 260], F32, tag="g")
                for c in range(3):
                    nc.sync.dma_start(
                        out=g[:nj, :, c, :],
                        in_=_ap(stg1, c * 136 * 260 + j0 * 260,
                                [[260, nj], [260, 5], [1, 260]]))
                center = g[:nj, 2, :, 2:258]
                ws = pB.tile([128, 3, 256], F32, tag="ws")
                wp = pB.tile([128, 3, 256], F32, tag="wp")
                nc.gpsimd.memset(ws[:nj], 1.0 + 1e-8)
                nc.vector.tensor_copy(out=wp[:nj], in_=center)
                for ki in range(5):
                    for kj in range(5):
                        if ki == 2 and kj == 2:
                            continue
                        patch = g[:nj, ki, :, kj:kj + 256]
                        d = pB.tile([128, 3, 256], F32, tag="d")
                        nc.vector.tensor_sub(out=d[:nj], in0=patch, in1=center)
                        sq = pB.tile([128, 3, 256], F32, tag="sq")
                        nc.scalar.activation(sq[:nj], d[:nj], AF.Square)
                        e = pB.tile([128, 3, 256], F32, tag="e")
                        nc.scalar.activation(e[:nj], sq[:nj], AF.Exp,
                                             bias=lnwt[:nj, lncol[float(lnw[ki, kj])]:
                                                       lncol[float(lnw[ki, kj])] + 1],
                                             scale=-float(inv2sig))
                        nc.vector.tensor_add(out=ws[:nj], in0=ws[:nj], in1=e[:nj])
                        ep = pB.tile([128, 3, 256], F32, tag="ep")
                        nc.vector.tensor_mul(out=ep[:nj], in0=e[:nj], in1=patch)
                        nc.vector.tensor_add(out=wp[:nj], in0=wp[:nj], in1=ep[:nj])
                rec = pB.tile([128, 3, 256], F32, tag="rec")
                nc.vector.reciprocal(rec[:nj], ws[:nj])
                xg = pB.tile([128, 3, 260], F32, tag="xg")
                nc.vector.memset(xg[:nj, :, 0:2], 0.0)
                nc.vector.memset(xg[:nj, :, 258:260], 0.0)
                xgi = xg[:nj, :, 2:258]
                nc.vector.tensor_mul(out=xgi, in0=wp[:nj], in1=rec[:nj])
                nc.vector.tensor_scalar(out=xgi, in0=xgi, scalar1=1e-8,
                                        scalar2=1.0, op0=ALU.max, op1=ALU.min)
                nc.scalar.activation(xgi, xgi, AF.Ln)
                nc.scalar.activation(xgi, xgi, AF.Exp, scale=float(invgam))
                if j0 == 0:
                    nc.sync.dma_start(
                        out=_ap(xlr_o, 0, [[256, 128], [128 * 256, 3], [1, 256]]),
                        in_=xg[:128, :, 2:258])
                nc.sync.dma_start(
                    out=_ap(stg2, (2 + j0) * 260, [[260, nj], [134 * 260, 3], [1, 260]]),
                    in_=xg[:nj])

        # ---------------- stage C: convs ----------------
        with (
            tc.tile_pool(name="pC", bufs=1) as pC,
            tc.tile_pool(name="pC2", bufs=2) as pC2,
            tc.tile_pool(name="psC", bufs=1, space="PSUM") as psC,
        ):
            w1t = pk.tile([15, 5 * 128], F32)
            nc.sync.dma_start(out=w1t, in_=_ap(T["w1p"], 0, [[128, 15], [15 * 128, 5], [1, 128]]))
            w2pt = pk.tile([128, 3 * 128], F32)
            nc.sync.dma_start(out=w2pt, in_=_ap(T["w2p"], 0, [[128, 128], [128 * 128, 3], [1, 128]]))
            w2st = pk.tile([64, 3 * 128], F32)
            nc.sync.dma_start(out=w2st, in_=_ap(T["w2s"], 0, [[128, 64], [64 * 128, 3], [1, 128]]))
            w3pt = pk.tile([128, 3 * 12], F32)
            nc.sync.dma_start(out=w3pt, in_=_ap(T["w3p"], 0, [[12, 128], [128 * 12, 3], [1, 12]]))
            w3st = pk.tile([64, 3 * 12], F32)
            nc.sync.dma_start(out=w3st, in_=_ap(T["w3s"], 0, [[12, 64], [64 * 12, 3], [1, 12]]))
            w3r0t = pk.tile([64, 12 * 6], F32)
            nc.sync.dma_start(out=w3r0t, in_=_ap(T["w3r0"], 0, [[6, 64], [64 * 6, 12], [1, 6]]))
            w3c0t = pk.tile([64, 16 * 6], F32)
            nc.sync.dma_start(out=w3c0t, in_=_ap(T["w3c0"], 0, [[6, 64], [64 * 6, 16], [1, 6]]))
            w3c1t = pk.tile([64, 16 * 6], F32)
            nc.sync.dma_start(out=w3c1t, in_=_ap(T["w3c1"], 0, [[6, 64], [64 * 6, 16], [1, 6]]))
            b1t = pk.tile([128, 1], F32)
            nc.sync.dma_start(out=b1t, in_=T["b1d"])
            b2t = pk.tile([128, 1], F32)
            nc.sync.dma_start(out=b2t, in_=T["b2d"])
            b3mt = pk.tile([12, 1], F32)
            nc.sync.dma_start(out=b3mt, in_=T["b3m"])
            b3rt = pk.tile([6, 1], F32)
            nc.sync.dma_start(out=b3rt, in_=T["b3r"])
            b3ct = pk.tile([6, 1], F32)
            nc.sync.dma_start(out=b3ct, in_=T["b3c"])

            y1t = pC.tile([128, 38, 258], F32)
            y2t = pC.tile([128, 36, 258], F32)
            y2b = pC.tile([64, 130, 6], F32)
            y2r = pC.tile([64, 3, 258], F32)
            # zero-init band tiles once: covers persistent zero pads of y1t
            # (cols 0/257, band-0 zero row t=1) and benign clamp-copy reads
            nc.vector.memset(y1t, 0.0)
            nc.vector.memset(y2t, 0.0)

            for j0 in BANDS:
                band0 = (j0 == 0)
                h_lo = 0 if band0 else j0 - 2
                h_hi = j0 + (36 if band0 else 34)
                nrows = h_hi - h_lo
                c1in = pC2.tile([15, 36, 260], F32, tag="c1in")
                for c in range(3):
                    nc.sync.dma_start(
                        out=c1in[c * 5:(c + 1) * 5, :nrows, :],
                        in_=_ap(stg2, c * 134 * 260 + h_lo * 260,
                                [[260, 5], [260, nrows], [1, 260]]))
                # conv1
                for r0 in range(h_lo, h_hi, 2):
                    pm = psC.tile([128, 2, 256], F32, tag="ps1", bufs=2)
                    u = r0 - h_lo
                    for dx in range(5):
                        nc.tensor.matmul(pm, w1t[:, dx * 128:(dx + 1) * 128],
                                         c1in[:, u:u + 2, dx:dx + 256],
                                         start=(dx == 0), stop=(dx == 4))
                    t0 = r0 - j0 + 2
                    nc.scalar.activation(y1t[0:64, t0:t0 + 2, 1:257], pm[0:64],
                                         AF.Relu, bias=b1t[0:64])
                    nc.scalar.activation(y1t[64:128, t0:t0 + 2, 0:256], pm[64:128],
                                         AF.Relu, bias=b1t[64:128])
                # conv2: band0 computes y2 rows [0,34); others [j0-1, j0+33)
                r_lo = 0 if band0 else j0 - 1
                for r0 in range(r_lo, r_lo + 34, 2):
                    pm2 = psC.tile([128, 2, 256], F32, tag="ps2", bufs=2)
                    for dy in range(3):
                        trow = r0 + dy + 1 - j0
                        nc.tensor.matmul(pm2, w2pt[:, dy * 128:(dy + 1) * 128],
                                         y1t[:, trow:trow + 2, 0:256],
                                         start=(dy == 0), stop=False)
                        nc.tensor.matmul(pm2, w2st[:, dy * 128:(dy + 1) * 128],
                                         y1t[0:64, trow:trow + 2, 2:258],
                                         start=False, stop=(dy == 2))
                    t2 = r0 - j0 + 1
                    nc.scalar.activation(y2t[0:64, t2:t2 + 2, 1:257], pm2[0:64],
                                         AF.Relu, bias=b2t[0:64])
                    nc.scalar.activation(y2t[64:128, t2:t2 + 2, 0:256], pm2[64:128],
                                         AF.Relu, bias=b2t[64:128])
                # y2t border cols (written rows only) + band0 clamp row
                ta, tb = (1, 35) if band0 else (0, 34)
                nc.vector.tensor_copy(out=y2t[0:64, ta:tb, 0:1], in_=y2t[0:64, ta:tb, 1:2])
                nc.vector.tensor_copy(out=y2t[0:64, ta:tb, 257:258], in_=y2t[0:64, ta:tb, 256:257])
                if band0:
                    nc.vector.tensor_copy(out=y2t[:, 0:1, :], in_=y2t[:, 1:2, :])
                # persist border data for fixes
                nc.vector.tensor_copy(out=y2b[:, j0:j0 + 34, 0:3],
                                      in_=y2t[0:64, 0:34, 0:3])
                nc.vector.tensor_copy(out=y2b[:, j0:j0 + 34, 3:6],
                                      in_=y2t[0:64, 0:34, 255:258])
                if band0:
                    nc.vector.tensor_copy(out=y2r, in_=y2t[0:64, 0:3, :])
                # fused conv3
                for j in range(j0, j0 + 32, 2):
                    pm3 = psC.tile([12, 2, 256], F32, tag="ps3", bufs=2)
                    for dy in range(3):
                        trow = j - j0 + dy
                        nc.tensor.matmul(pm3, w3pt[:, dy * 12:(dy + 1) * 12],
                                         y2t[:, trow:trow + 2, 0:256],
                                         start=(dy == 0), stop=False)
                        nc.tensor.matmul(pm3, w3st[:, dy * 12:(dy + 1) * 12],
                                         y2t[0:64, trow:trow + 2, 2:258],
                                         start=False, stop=(dy == 2))
                    xsb = pC2.tile([12, 2, 256], F32, tag="xsb")
                    nc.scalar.activation(xsb, pm3, AF.Identity, bias=b3mt)
                    nc.sync.dma_start(
                        out=_ap(xsr_o, j * 256, [[32768, 12], [256, 2], [1, 256]]),
                        in_=xsb)

            # ---------------- border fixes ----------------
            pr = psC.tile([6, 256], F32, tag="psf")
            i = 0
            for dy in range(3):
                for dx in range(3):
                    nc.tensor.matmul(pr, w3r0t[:, i * 6:(i + 1) * 6],
                                     y2r[:, dy, dx:dx + 256],
                                     start=(i == 0), stop=False)
                    i += 1
            for dx in range(3):
                nc.tensor.matmul(pr, w3r0t[:, i * 6:(i + 1) * 6],
                                 y2r[:, 1, dx:dx + 256],
                                 start=False, stop=(i == 11))
                i += 1
            xrb = pC.tile([6, 256], F32)
            nc.scalar.activation(xrb, pr, AF.Identity, bias=b3rt)

            def col_psum(wt, cols, ccol):
                pc = psC.tile([6, 128], F32, tag="psf")
                i = 0
                for dy in range(3):
                    for dx in range(3):
                        rhs = _ap(y2b, dy * 6 + cols[dx], [[y2b.ap[0][0], 64], [6, 128]])
                        nc.tensor.matmul(pc, wt[:, i * 6:(i + 1) * 6], rhs,
                                         start=(i == 0), stop=False)
                        i += 1
                for dx in range(3):
                    rhs = _ap(y2b, 1 * 6 + cols[dx], [[y2b.ap[0][0], 64], [6, 1]])
                    nc.tensor.matmul(pc[0:3, 0:1], wt[:, (12 + dx) * 6:(12 + dx) * 6 + 3],
                                     rhs, start=False, stop=False, skip_group_check=True)
                rhs = _ap(y2b, 1 * 6 + ccol, [[y2b.ap[0][0], 64], [6, 1]])
                nc.tensor.matmul(pc[0:3, 0:1], wt[:, 15 * 6:15 * 6 + 3], rhs,
                                 start=False, stop=False, skip_group_check=True)
                for dy in range(3):
                    rhs = _ap(y2b, dy * 6 + ccol, [[y2b.ap[0][0], 64], [6, 128]])
                    nc.tensor.matmul(pc, wt[:, (9 + dy) * 6:(10 + dy) * 6], rhs,
                                     start=False, stop=(dy == 2))
                out = pC.tile([6, 128], F32, tag=f"xcb{ccol}")
                nc.scalar.activation(out, pc, AF.Identity, bias=b3ct)
                return out

            xc0 = col_psum(w3c0t, [0, 1, 2], 1)
            xc1 = col_psum(w3c1t, [3, 4, 5], 4)

            tc.strict_bb_all_engine_barrier()
            # row 0: px=0 cols 1..255, px=1 cols 0..254  (corners via col fixes)
            nc.sync.dma_start(out=_ap(xsr_o, 1, [[131072, 3], [1, 255]]),
                              in_=xrb[0:3, 1:256])
            nc.sync.dma_start(out=_ap(xsr_o, 32768, [[131072, 3], [1, 255]]),
                              in_=xrb[3:6, 0:255])
            for py in range(2):
                nc.sync.dma_start(
                    out=_ap(xsr_o, py * 65536, [[131072, 3], [256, 128]]),
                    in_=xc0[py * 3:(py + 1) * 3, :])
                nc.sync.dma_start(
                    out=_ap(xsr_o, py * 65536 + 32768 + 255, [[131072, 3], [256, 128]]),
                    in_=xc1[py * 3:(py + 1) * 3, :])


# --------------------------------------------------------------------------
# host entry
# --------------------------------------------------------------------------

_PROG_CACHE = {}


def _get_program(sigma_color, gamma):
    key = (float(sigma_color), float(gamma))
    if key not in _PROG_CACHE:
        _PROG_CACHE[key] = build_program(sigma_color, gamma)
    return _PROG_CACHE[key]


def _in_maps(x_hr, gains, sigma_color, gamma, w1, b1, w2, b2, w3, b3):
    packs = [host_weights(gains, w1, b1, w2, b2, w3, b3, flip) for flip in (0, 1)]
    maps = []
    for core in range(8):
        b, half = core // 2, core % 2
        m = {"slab": make_slab(np.asarray(x_hr[b], np.float32), half)}
        for k, v in packs[half].items():
            m[k] = np.ascontiguousarray(v, np.float32)
        maps.append(m)
    return maps


def _assemble(results, x_hr_dtype):
    x_sr = np.zeros((4, 3, 512, 512), np.float32)
    x_lr = np.zeros((4, 3, 256, 256), np.float32)
    for core in range(8):
        b, half = core // 2, core % 2
        sr = results[core]["xsr"]          # [3,2,2,128,256] parity planar
        lr = results[core]["xlr"]          # [3,128,256]
        full = np.zeros((3, 256, 512), np.float32)
        for py in range(2):
            for px in range(2):
                full[:, py::2, px::2] = sr[:, py, px]
        if half == 0:
            x_sr[b, :, :256] = full
            x_lr[b, :, :128] = lr
        else:
            x_sr[b, :, 256:] = full[:, ::-1, :]
            x_lr[b, :, 128:] = lr[:, ::-1, :]
    return x_sr, x_lr


def kernel(x_hr, gains, sigma_color, gamma, w1, b1, w2, b2, w3, b3):
    import os
    nc = _get_program(float(sigma_color), float(gamma))
    maps = _in_maps(x_hr, gains, sigma_color, gamma, w1, b1, w2, b2, w3, b3)
    trace = bool(os.environ.get("KPROF"))
    res = bass_utils.run_bass_kernel_spmd(nc, maps, core_ids=list(range(8)),
                                          trace=trace)
    if trace:
        print(f"HW exec time: {res.exec_time_ns} ns")
    x_sr, x_lr = _assemble(res.results, np.asarray(x_hr).dtype)
    return x_sr, x_lr


# --------------------------------------------------------------------------
# self-test (CoreSim on core 0) -- not used by the grader
# --------------------------------------------------------------------------

if __name__ == "__main__":
    from concourse.bass_interp import CoreSim
    data = dict(np.load("/tmp/inputs.npz"))
    nc = build_program(float(data["sigma_color"]), float(data["gamma"]))
    maps = _in_maps(**data)
    core = int(sys.argv[1]) if len(sys.argv) > 1 else 0
    sim = CoreSim(nc, trace=False, require_finite=False, require_nnan=False)
    for k, v in maps[core].items():
        sim.tensor(k)[:] = v
    sim.simulate(check_with_hw=False)
    sr = np.asarray(sim.tensor("xsr"))
    lr = np.asarray(sim.tensor("xlr"))
    import prototype as P
    b, half = core // 2, core % 2
    exp_sr, exp_lr = P.run_core(data["x_hr"][b], half, data)
    full = np.zeros((3, 256, 512), np.float32)
    for py in range(2):
        for px in range(2):
            full[:, py::2, px::2] = sr[:, py, px]
    if half == 1:
        pass  # compare in core orientation: exp_sr already core-oriented pre-flip
    e1 = np.abs(full - (exp_sr if half == 0 else exp_sr[:, ::-1, :])).max()
    e2 = np.abs(lr - (exp_lr if half == 0 else exp_lr[:, ::-1, :])).max()
    print(f"core {core}: xsr err {e1:.3e}  xlr err {e2:.3e}")
